# revision 5
# baseline (speedup 1.0000x reference)
"""Grouped SwiGLU FFN (8 experts) — expert-parallel Bass kernel for 8 trn2 cores.

Per core (one expert): out = (silu(x@w1) * (x@w3T)) @ w2T, all fp32.
  x: [T=1024, D=2048], w1: [D, H=4096], w3: [H, D], w2: [D, H].

Device-side formulation (all matmuls in float32r at full PE rate, zero
on-device transposes — layouts are pre-packed on host):
  phase1: g^T[h, t]  = silu(w1^T-tile.T @ x^T) * (w3-tile.T @ x^T)   (per h-tile)
  phase2: out^T[d,t] = sum_h w2-tile.T @ g^T                          (w2 stationary)
H is processed in 8 slices of 512 (4 h-tiles); out^T accumulated in SBUF fp32.
Host unpacks outT -> out.
"""

import sys

sys.path.insert(0, "/opt/trn_rl_repo")

import numpy as np

import concourse.bass as bass
from concourse import bacc
import concourse.mybir as mybir
import concourse.tile as tile
from concourse.bass_utils import run_bass_kernel_spmd

E, T, D, H = 8, 1024, 2048, 4096
P = 128
NT = 512            # matmul moving free dim (fp32 max)
DT = D // P         # 16 contraction tiles over D
HT = H // P         # 32 h-tiles
HQ = 8              # h-phases
HTQ = HT // HQ      # 4 h-tiles per phase
TH = T // NT        # 2 t-halves
DTT = D // P        # 16 out^T row tiles
F32 = mybir.dt.float32
F32R = mybir.dt.float32r

_CACHE: dict = {}


def _build_nc():
    nc = bacc.Bacc("TRN2", target_bir_lowering=False, debug=False)
    xp = nc.dram_tensor("xp", [DT, P, T], F32R, kind="ExternalInput")
    w1p = nc.dram_tensor("w1p", [HT, P, DT, P], F32R, kind="ExternalInput")
    w3p = nc.dram_tensor("w3p", [HT, P, DT, P], F32R, kind="ExternalInput")
    w2p = nc.dram_tensor("w2p", [HQ, DTT, P, HTQ, P], F32R, kind="ExternalInput")
    outT = nc.dram_tensor("outT", [D, T], F32, kind="ExternalOutput")

    with tile.TileContext(nc) as tc:
        with (
            tc.tile_pool(name="xpool", bufs=1) as xpool,
            tc.tile_pool(name="gpool", bufs=1) as gpool,
            tc.tile_pool(name="opool", bufs=1) as opool,
            tc.tile_pool(name="wpool", bufs=2) as wpool,
            tc.tile_pool(name="w2pool", bufs=2) as w2pool,
            tc.tile_pool(name="spool", bufs=2) as spool,
            tc.tile_pool(name="pspool", bufs=2, space="PSUM") as pspool,
            tc.tile_pool(name="popool", bufs=4, space="PSUM") as popool,
        ):
            xsb = xpool.tile([P, DT, T], F32R, tag="x")
            for dt_i in range(DT):
                nc.sync.dma_start(xsb[:, dt_i], xp[dt_i])
            out_acc = opool.tile([P, DTT, T], F32, tag="oacc")

            for hq in range(HQ):
                g = gpool.tile([P, HTQ, T], F32R, tag="g")
                for htl in range(HTQ):
                    ht = hq * HTQ + htl
                    w1sb = wpool.tile([P, DT, P], F32R, tag="w1")
                    nc.sync.dma_start(w1sb, w1p[ht])
                    w3sb = wpool.tile([P, DT, P], F32R, tag="w3")
                    nc.sync.dma_start(w3sb, w3p[ht])
                    for th in range(TH):
                        ts = slice(th * NT, (th + 1) * NT)
                        ps1 = pspool.tile([P, NT], F32, tag="ps1")
                        ps3 = pspool.tile([P, NT], F32, tag="ps3")
                        for dt_i in range(DT):
                            nc.tensor.matmul(
                                ps1,
                                lhsT=w1sb[:, dt_i],
                                rhs=xsb[:, dt_i, ts],
                                start=(dt_i == 0),
                                stop=(dt_i == DT - 1),
                            )
                        for dt_i in range(DT):
                            nc.tensor.matmul(
                                ps3,
                                lhsT=w3sb[:, dt_i],
                                rhs=xsb[:, dt_i, ts],
                                start=(dt_i == 0),
                                stop=(dt_i == DT - 1),
                            )
                        sg = spool.tile([P, NT], F32, tag="sg")
                        nc.scalar.activation(
                            sg, ps1, mybir.ActivationFunctionType.Sigmoid
                        )
                        sil = spool.tile([P, NT], F32, tag="sil")
                        nc.vector.tensor_mul(out=sil, in0=sg, in1=ps1)
                        nc.vector.tensor_mul(out=g[:, htl, ts], in0=sil, in1=ps3)

                for dtt in range(DTT):
                    w2sb = w2pool.tile([P, HTQ, P], F32R, tag="w2")
                    nc.sync.dma_start(w2sb, w2p[hq, dtt])
                    for th in range(TH):
                        ts = slice(th * NT, (th + 1) * NT)
                        po = popool.tile([P, NT], F32, tag="po")
                        for htl in range(HTQ):
                            nc.tensor.matmul(
                                po,
                                lhsT=w2sb[:, htl],
                                rhs=g[:, htl, ts],
                                start=(htl == 0),
                                stop=(htl == HTQ - 1),
                            )
                        if hq == 0:
                            nc.vector.tensor_copy(out=out_acc[:, dtt, ts], in_=po)
                        else:
                            nc.vector.tensor_add(
                                out=out_acc[:, dtt, ts],
                                in0=out_acc[:, dtt, ts],
                                in1=po,
                            )

            for dtt in range(DTT):
                nc.sync.dma_start(outT[dtt * P : (dtt + 1) * P, :], out_acc[:, dtt])
    nc.compile()
    return nc


def _round_fp32r(a):
    """Round fp32 to the fp32r grid: 11 explicit mantissa bits (low 12 bits
    zero), round-to-nearest-even — what the PE consumes at full rate."""
    u = np.ascontiguousarray(a, dtype=np.float32).view(np.uint32)
    low = u & np.uint32(0xFFF)
    base = u & np.uint32(0xFFFFF000)
    lsb = (base >> np.uint32(12)) & np.uint32(1)
    roundup = (low > 0x800) | ((low == 0x800) & (lsb == 1))
    out = base + (roundup.astype(np.uint32) << np.uint32(12))
    return out.view(np.float32)


def _pack_inputs(x, w1, w2, w3):
    """Per-expert host-side packing into DMA-linear layouts."""
    in_maps = []
    for e in range(E):
        xe = _round_fp32r(np.asarray(x[e], dtype=np.float32))
        w1e = _round_fp32r(np.asarray(w1[e], dtype=np.float32))
        w2e = _round_fp32r(np.asarray(w2[e], dtype=np.float32))
        w3e = _round_fp32r(np.asarray(w3[e], dtype=np.float32))
        # xp[dt, p, t] = x[t, dt*128+p]
        xp = np.ascontiguousarray(xe.reshape(T, DT, P).transpose(1, 2, 0))
        # w1p[ht, p, dt, h] = w1[dt*128+p, ht*128+h]
        w1p = np.ascontiguousarray(
            w1e.reshape(DT, P, HT, P).transpose(2, 1, 0, 3)
        )
        # w3p[ht, p, dt, h] = w3[ht*128+h, dt*128+p]
        w3p = np.ascontiguousarray(
            w3e.reshape(HT, P, DT, P).transpose(0, 3, 2, 1)
        )
        # w2p[hq, dtt, p, htl, d] = w2[dtt*128+d, (hq*HTQ+htl)*128+p]
        w2p = np.ascontiguousarray(
            w2e.reshape(DTT, P, HQ, HTQ, P).transpose(2, 0, 4, 3, 1)
        )
        in_maps.append({"xp": xp, "w1p": w1p, "w3p": w3p, "w2p": w2p})
    return in_maps


def kernel(x, w1, w2, w3, _trace=False, _trace_kwargs=None):
    if "nc" not in _CACHE:
        _CACHE["nc"] = _build_nc()
    nc = _CACHE["nc"]
    in_maps = _pack_inputs(x, w1, w2, w3)
    kw = {}
    if _trace:
        kw = {"trace": True}
        if _trace_kwargs:
            kw.update(_trace_kwargs)
    res = run_bass_kernel_spmd(nc, in_maps, core_ids=list(range(E)), **kw)
    out = np.empty((E, T, D), dtype=np.float32)
    for e in range(E):
        out[e] = res.results[e]["outT"].T
    if _trace:
        _CACHE["last_results"] = res
    return out


# revision 6
# speedup vs baseline: 1.0281x; 1.0281x over previous
"""Grouped SwiGLU FFN (8 experts) — expert-parallel Bass kernel for 8 trn2 cores.

Per core (one expert): out = (silu(x@w1) * (x@w3T)) @ w2T, all fp32.
  x: [T=1024, D=2048], w1: [D, H=4096], w3: [H, D], w2: [D, H].

Device-side formulation (all matmuls in float32r at full PE rate, zero
on-device transposes — layouts are pre-packed on host):
  phase1: g^T[h, t]  = silu(w1^T-tile.T @ x^T) * (w3-tile.T @ x^T)   (per h-tile)
  phase2: out^T[d,t] = sum_h w2-tile.T @ g^T                          (w2 stationary)
H is processed in 8 slices of 512 (4 h-tiles); out^T accumulated in SBUF fp32.
Host unpacks outT -> out.
"""

import sys

sys.path.insert(0, "/opt/trn_rl_repo")

import numpy as np

import concourse.bass as bass
from concourse import bacc
import concourse.mybir as mybir
import concourse.tile as tile
from concourse.bass_utils import run_bass_kernel_spmd

E, T, D, H = 8, 1024, 2048, 4096
P = 128
NT = 512            # matmul moving free dim (fp32 max)
DT = D // P         # 16 contraction tiles over D
HT = H // P         # 32 h-tiles
HQ = 4              # h-phases
HTQ = HT // HQ      # 4 h-tiles per phase
TH = T // NT        # 2 t-halves
DTT = D // P        # 16 out^T row tiles
F32 = mybir.dt.float32
F32R = mybir.dt.float32r

_CACHE: dict = {}
USE_SILU = True


def _build_nc():
    nc = bacc.Bacc("TRN2", target_bir_lowering=False, debug=False)
    xp = nc.dram_tensor("xp", [DT, P, T], F32R, kind="ExternalInput")
    w1p = nc.dram_tensor("w1p", [HT, P, DT, P], F32R, kind="ExternalInput")
    w3p = nc.dram_tensor("w3p", [HT, P, DT, P], F32R, kind="ExternalInput")
    w2p = nc.dram_tensor("w2p", [HQ, DTT, P, HTQ, P], F32R, kind="ExternalInput")
    outT = nc.dram_tensor("outT", [D, T], F32, kind="ExternalOutput")

    with tile.TileContext(nc) as tc:
        with (
            tc.tile_pool(name="xpool", bufs=1) as xpool,
            tc.tile_pool(name="gpool", bufs=1) as gpool,
            tc.tile_pool(name="opool", bufs=1) as opool,
            tc.tile_pool(name="wpool", bufs=2) as wpool,
            tc.tile_pool(name="w2pool", bufs=2) as w2pool,
            tc.tile_pool(name="spool", bufs=2) as spool,
            tc.tile_pool(name="pspool", bufs=2, space="PSUM") as pspool,
            tc.tile_pool(name="popool", bufs=4, space="PSUM") as popool,
        ):
            def load_w(ht):
                w1sb = wpool.tile([P, DT, P], F32R, tag="w1", name=f"w1sb_{ht}")
                nc.sync.dma_start(w1sb, w1p[ht])
                w3sb = wpool.tile([P, DT, P], F32R, tag="w3", name=f"w3sb_{ht}")
                nc.sync.dma_start(w3sb, w3p[ht])
                return w1sb, w3sb

            # first weight tiles before the bulk x load so PE starts ASAP
            w_pre = load_w(0)
            xsb = xpool.tile([P, DT, T], F32R, tag="x")
            for dt_i in range(DT):
                nc.sync.dma_start(xsb[:, dt_i], xp[dt_i])
            out_acc = opool.tile([P, DTT, T], F32, tag="oacc")

            for hq in range(HQ):
                g = gpool.tile([P, HTQ, T], F32R, tag="g")
                for htl in range(HTQ):
                    ht = hq * HTQ + htl
                    w1sb, w3sb = w_pre if ht == 0 else load_w(ht)
                    for th in range(TH):
                        ts = slice(th * NT, (th + 1) * NT)
                        ps1 = pspool.tile([P, NT], F32, tag="ps1")
                        ps3 = pspool.tile([P, NT], F32, tag="ps3")
                        for dt_i in range(DT):
                            nc.tensor.matmul(
                                ps1,
                                lhsT=w1sb[:, dt_i],
                                rhs=xsb[:, dt_i, ts],
                                start=(dt_i == 0),
                                stop=(dt_i == DT - 1),
                            )
                        for dt_i in range(DT):
                            nc.tensor.matmul(
                                ps3,
                                lhsT=w3sb[:, dt_i],
                                rhs=xsb[:, dt_i, ts],
                                start=(dt_i == 0),
                                stop=(dt_i == DT - 1),
                            )
                        sil = spool.tile([P, NT], F32, tag="sil")
                        if USE_SILU:
                            nc.scalar.activation(
                                sil, ps1, mybir.ActivationFunctionType.Silu
                            )
                        else:
                            sg = spool.tile([P, NT], F32, tag="sg")
                            nc.scalar.activation(
                                sg, ps1, mybir.ActivationFunctionType.Sigmoid
                            )
                            nc.vector.tensor_mul(out=sil, in0=sg, in1=ps1)
                        nc.vector.tensor_mul(out=g[:, htl, ts], in0=sil, in1=ps3)

                for dtt in range(DTT):
                    w2sb = w2pool.tile([P, HTQ, P], F32R, tag="w2")
                    nc.sync.dma_start(w2sb, w2p[hq, dtt])
                    for th in range(TH):
                        ts = slice(th * NT, (th + 1) * NT)
                        po = popool.tile([P, NT], F32, tag="po")
                        for htl in range(HTQ):
                            nc.tensor.matmul(
                                po,
                                lhsT=w2sb[:, htl],
                                rhs=g[:, htl, ts],
                                start=(htl == 0),
                                stop=(htl == HTQ - 1),
                            )
                        if hq == 0:
                            nc.vector.tensor_copy(out=out_acc[:, dtt, ts], in_=po)
                        else:
                            nc.vector.tensor_add(
                                out=out_acc[:, dtt, ts],
                                in0=out_acc[:, dtt, ts],
                                in1=po,
                            )

            for dtt in range(DTT):
                nc.sync.dma_start(outT[dtt * P : (dtt + 1) * P, :], out_acc[:, dtt])
    nc.compile()
    return nc


def _round_fp32r(a):
    """Round fp32 to the fp32r grid: 11 explicit mantissa bits (low 12 bits
    zero), round-to-nearest-even — what the PE consumes at full rate."""
    u = np.ascontiguousarray(a, dtype=np.float32).view(np.uint32)
    low = u & np.uint32(0xFFF)
    base = u & np.uint32(0xFFFFF000)
    lsb = (base >> np.uint32(12)) & np.uint32(1)
    roundup = (low > 0x800) | ((low == 0x800) & (lsb == 1))
    out = base + (roundup.astype(np.uint32) << np.uint32(12))
    return out.view(np.float32)


def _pack_inputs(x, w1, w2, w3):
    """Per-expert host-side packing into DMA-linear layouts."""
    in_maps = []
    for e in range(E):
        xe = _round_fp32r(np.asarray(x[e], dtype=np.float32))
        w1e = _round_fp32r(np.asarray(w1[e], dtype=np.float32))
        w2e = _round_fp32r(np.asarray(w2[e], dtype=np.float32))
        w3e = _round_fp32r(np.asarray(w3[e], dtype=np.float32))
        # xp[dt, p, t] = x[t, dt*128+p]
        xp = np.ascontiguousarray(xe.reshape(T, DT, P).transpose(1, 2, 0))
        # w1p[ht, p, dt, h] = w1[dt*128+p, ht*128+h]
        w1p = np.ascontiguousarray(
            w1e.reshape(DT, P, HT, P).transpose(2, 1, 0, 3)
        )
        # w3p[ht, p, dt, h] = w3[ht*128+h, dt*128+p]
        w3p = np.ascontiguousarray(
            w3e.reshape(HT, P, DT, P).transpose(0, 3, 2, 1)
        )
        # w2p[hq, dtt, p, htl, d] = w2[dtt*128+d, (hq*HTQ+htl)*128+p]
        w2p = np.ascontiguousarray(
            w2e.reshape(DTT, P, HQ, HTQ, P).transpose(2, 0, 4, 3, 1)
        )
        in_maps.append({"xp": xp, "w1p": w1p, "w3p": w3p, "w2p": w2p})
    return in_maps


def kernel(x, w1, w2, w3, _trace=False, _trace_kwargs=None):
    if "nc" not in _CACHE:
        _CACHE["nc"] = _build_nc()
    nc = _CACHE["nc"]
    in_maps = _pack_inputs(x, w1, w2, w3)
    kw = {}
    if _trace:
        kw = {"trace": True}
        if _trace_kwargs:
            kw.update(_trace_kwargs)
    res = run_bass_kernel_spmd(nc, in_maps, core_ids=list(range(E)), **kw)
    out = np.empty((E, T, D), dtype=np.float32)
    for e in range(E):
        out[e] = res.results[e]["outT"].T
    if _trace:
        _CACHE["last_results"] = res
    return out
